# revision 7
# baseline (speedup 1.0000x reference)
"""Trainium2 Bass kernel for nn_CTDAutoEncoder (VQ autoencoder forward).

Self-contained: kernel(**inputs) takes full inputs, shards batch 64 -> 8 cores
(8 samples each), runs one SPMD Bass kernel, gathers full output [64,32,32,16].

Device layout: activations channel-major [C(partitions), pix/tok(free)], C=256 ->
2 partition-tiles of 128. Convs = shifted-AP matmuls; BN folded into weights;
embed folded into enc conv1 (contraction over vocab=16); convT via parity classes;
LN stats via ones-matmul partition reduction; VQ argmin via DVE top-8 max of
s = 2*z.c - |c|^2 (z stationary -> scores token-major), onehot-matmul gather.
"""
import sys
sys.path.insert(0, '/opt/trn_rl_repo')
import contextlib
import numpy as np

import concourse.bass as bass
import concourse.tile as tile
from concourse import mybir, bacc
from concourse.bass_utils import run_bass_kernel_spmd
from concourse.masks import make_identity

F32 = mybir.dt.float32
F32R = mybir.dt.float32r
I32 = mybir.dt.int32
AF = mybir.ActivationFunctionType
ALU = mybir.AluOpType

C, NV, NH, NL = 256, 16, 4, 3
BNS = 1.0 / np.sqrt(1.0 + 1e-5)
NSAMP = 8
NTOK = NSAMP * 64

# per-stage matmul dtype: F32 (exact, 4cyc/row) or F32R (fast, reduced precision)
DT_ENC = F32
DT_VQ = F32
DT_Q = F32
DT_DEC = F32
DT_LG = F32
DT_STAT = F32

TAPS9 = [(ky, kx) for ky in range(3) for kx in range(3)]

# ---------------------------------------------------------------------------
# weight layout (shared host/builder)
# ---------------------------------------------------------------------------

def _wdefs():
    d = []
    for t in range(9):
        d.append(("s1a", f"e1c1_t{t}", NV, C))
    d.append(("s1a", "e1sc", NV, C))
    for t in range(9):
        d.append(("s1b", f"e1c2_t{t}", C, C))
    for t in range(9):
        d.append(("s2a", f"e2c1_t{t}", C, C))
    d.append(("s2a", "e2sc", C, C))
    for t in range(9):
        d.append(("s2b", f"e2c2_t{t}", C, C))
    for pre in ("te", "td"):
        for i in range(NL):
            g = f"{pre}{i}"
            d.append((g, f"{g}_qkv", C, 3 * C))
            d.append((g, f"{g}_wo", C, C))
            d.append((g, f"{g}_w1", C, 4 * C))
            d.append((g, f"{g}_w2", 4 * C, C))
    d.append(("vq", "vq_rhs", C, 1024))
    d.append(("vq", "cb", 1024, C))
    for t in range(9):
        d.append(("d1a", f"d1c1_t{t}", C, C))
    d.append(("d1a", "d1sc", C, C))
    for t in range(9):
        d.append(("d1b", f"d1c2_t{t}", C, C))
    for t in range(9):
        d.append(("d2a", f"d2c1_t{t}", C, C))
    d.append(("d2a", "d2sc", C, C))
    for t in range(9):
        d.append(("d2b", f"d2c2_t{t}", C, C))
    d.append(("d2b", "outw", C, NV))
    d.append(("pers", "pos", C, NTOK))
    return d


def _wlayout():
    groups, names, order = {}, {}, []
    for g, n, K, M in _wdefs():
        if g not in groups:
            groups[g] = 0
            order.append(g)
        kt = (K + 127) // 128
        names[n] = (g, groups[g], kt, M, K)
        groups[g] += kt * M
    goff, off = {}, 0
    for g in order:
        goff[g] = (off, groups[g])
        off += groups[g]
    return off, goff, names


WTOT, GOFF, WNAMES = _wlayout()

BIAS_NAMES = ["e1c1_b", "e1sc_b", "e1c2_b", "e2c1_b", "e2sc_b", "e2c2_b",
              "enc_ln_g", "enc_ln_b", "t_enc_ln_g", "t_enc_ln_b",
              "d1c1_b", "d1sc_b", "d1c2_b", "d2c1_b", "d2sc_b", "d2c2_b"]
BOFF = {n: 2 * i for i, n in enumerate(BIAS_NAMES)}
BOFF["out_b"] = 2 * len(BIAS_NAMES)
NBIAS = 2 * len(BIAS_NAMES) + 1


# ---------------------------------------------------------------------------
# host-side param prep
# ---------------------------------------------------------------------------

def _fold_bn(w, b, g, be, out_axis=0):
    s = (np.asarray(g, np.float64) * BNS)
    w = np.asarray(w, np.float64)
    shape = [1] * w.ndim
    shape[out_axis] = -1
    return (w * s.reshape(shape)).astype(np.float32), \
           (np.asarray(b, np.float64) * s + np.asarray(be, np.float64)).astype(np.float32)


def prep_arrays(params, codebook):
    A = {}
    E = np.asarray(params['embed'], np.float32)

    p0 = params['enc_blocks'][0]
    w1, b1 = _fold_bn(p0['w1'], p0['b1'], p0['g1'], p0['be1'])
    for t, (ky, kx) in enumerate(TAPS9):
        A[f'e1c1_t{t}'] = np.ascontiguousarray(E @ w1[:, :, ky, kx].T)
    A['e1c1_b'] = b1
    ws, bs = _fold_bn(p0['ws'], p0['bs'], p0['gs'], p0['bes'])
    A['e1sc'] = np.ascontiguousarray(E @ ws[:, :, 0, 0].T)
    A['e1sc_b'] = bs
    w2, b2 = _fold_bn(p0['w2'], p0['b2'], p0['g2'], p0['be2'])
    for t, (ky, kx) in enumerate(TAPS9):
        A[f'e1c2_t{t}'] = np.ascontiguousarray(w2[:, :, ky, kx].T)
    A['e1c2_b'] = b2

    p1 = params['enc_blocks'][1]
    w1, b1 = _fold_bn(p1['w1'], p1['b1'], p1['g1'], p1['be1'])
    for t, (ky, kx) in enumerate(TAPS9):
        A[f'e2c1_t{t}'] = np.ascontiguousarray(w1[:, :, ky, kx].T)
    A['e2c1_b'] = b1
    ws, bs = _fold_bn(p1['ws'], p1['bs'], p1['gs'], p1['bes'])
    A['e2sc'] = np.ascontiguousarray(ws[:, :, 0, 0].T)
    A['e2sc_b'] = bs
    w2, b2 = _fold_bn(p1['w2'], p1['b2'], p1['g2'], p1['be2'])
    for t, (ky, kx) in enumerate(TAPS9):
        A[f'e2c2_t{t}'] = np.ascontiguousarray(w2[:, :, ky, kx].T)
    A['e2c2_b'] = b2

    A['enc_ln_g'] = np.asarray(params['enc_ln_g'], np.float32)
    A['enc_ln_b'] = np.asarray(params['enc_ln_b'], np.float32)
    A['t_enc_ln_g'] = np.asarray(params['t_enc_ln_g'], np.float32)
    A['t_enc_ln_b'] = np.asarray(params['t_enc_ln_b'], np.float32)
    pos = np.asarray(params['pos'], np.float32)[0]
    A['pos'] = np.ascontiguousarray(np.tile(pos.T, (1, NSAMP)))

    for pre, layers in (('te', params['t_enc']), ('td', params['t_dec'])):
        for i, tp in enumerate(layers):
            g1 = np.asarray(tp['ln1'], np.float32)
            wqkv = np.asarray(tp['wqkv'], np.float32)
            A[f'{pre}{i}_qkv'] = np.ascontiguousarray((wqkv * g1[None, :]).T)
            A[f'{pre}{i}_wo'] = np.ascontiguousarray(np.asarray(tp['wo'], np.float32).T)
            g2 = np.asarray(tp['ln2'], np.float32)
            A[f'{pre}{i}_w1'] = np.ascontiguousarray((np.asarray(tp['w1'], np.float32) * g2[None, :]).T)
            A[f'{pre}{i}_w2'] = np.ascontiguousarray(np.asarray(tp['w2'], np.float32).T)

    cb = np.asarray(codebook, np.float32)
    A['vq_rhs'] = np.ascontiguousarray(2.0 * cb.T)
    A['vq_c2'] = -(cb.astype(np.float64) ** 2).sum(-1).astype(np.float32)[None]
    A['cb'] = cb

    for pre, pd in (('d1', params['dec_blocks'][0]), ('d2', params['dec_blocks'][1])):
        w1 = np.asarray(pd['w1'], np.float64)
        s = np.asarray(pd['g1'], np.float64) * BNS
        w1 = (w1 * s[None, :, None, None]).astype(np.float32)
        b1 = (np.asarray(pd['b1'], np.float64) * s + np.asarray(pd['be1'], np.float64)).astype(np.float32)
        for dy in range(3):
            for dx in range(3):
                A[f'{pre}c1_t{dy * 3 + dx}'] = np.ascontiguousarray(w1[:, :, 2 - dy, 2 - dx])
        A[f'{pre}c1_b'] = b1
        ws = np.asarray(pd['ws'], np.float64)
        ss = np.asarray(pd['gs'], np.float64) * BNS
        A[f'{pre}sc'] = np.ascontiguousarray((ws[:, :, 0, 0] * ss[None, :]).astype(np.float32))
        A[f'{pre}sc_b'] = (np.asarray(pd['bs'], np.float64) * ss + np.asarray(pd['bes'], np.float64)).astype(np.float32)
        w2, b2 = _fold_bn(pd['w2'], pd['b2'], pd['g2'], pd['be2'])
        for t, (ky, kx) in enumerate(TAPS9):
            A[f'{pre}c2_t{t}'] = np.ascontiguousarray(w2[:, :, ky, kx].T)
        A[f'{pre}c2_b'] = b2

    A['outw'] = np.ascontiguousarray(np.asarray(params['out_w'], np.float32).T)
    A['out_b'] = np.asarray(params['out_b'], np.float32)
    return A


def pack_host(A):
    wb = np.zeros((128, WTOT), np.float32)
    for n, (g, noff, kt, M, K) in WNAMES.items():
        off = GOFF[g][0] + noff
        arr = A[n]
        assert arr.shape == (K, M), (n, arr.shape, K, M)
        for k in range(kt):
            rows = min(128, K - k * 128)
            wb[0:rows, off + k * M: off + (k + 1) * M] = arr[k * 128:k * 128 + rows]
    bb = np.zeros((128, NBIAS), np.float32)
    for n in BIAS_NAMES:
        arr = A[n]
        bb[:, BOFF[n]] = arr[0:128]
        bb[:, BOFF[n] + 1] = arr[128:256]
    bb[0:16, BOFF['out_b']] = A['out_b']
    return wb, bb, np.ascontiguousarray(A['vq_c2'])


# ---------------------------------------------------------------------------
# device kernel builder
# ---------------------------------------------------------------------------

def vap(t, off, dims):
    """manual free-dim AP view of a (possibly partition-sliced) tile AP"""
    return bass.AP(tensor=t.tensor, offset=t.offset + off,
                   ap=[list(t.ap[0])] + [list(d) for d in dims])


class KB:
    def __init__(self):
        self.nc = bacc.Bacc()
        nc = self.nc
        self.wb = nc.declare_dram_parameter("wb", [128, WTOT], F32, isOutput=False)
        self.bbp = nc.declare_dram_parameter("bb", [128, NBIAS], F32, isOutput=False)
        self.c2p = nc.declare_dram_parameter("c2", [1, 1024], F32, isOutput=False)
        self.tokp = nc.declare_dram_parameter("tok", [1, NSAMP * 1024], I32, isOutput=False)
        self.outp = nc.declare_dram_parameter("out", [NSAMP * 1024, NV], F32, isOutput=True)

    def cast(self, ap, dt):
        return ap.bitcast(dt) if dt is not F32 else ap

    def mm(self, ps, pairs, dt):
        nc = self.nc
        n = len(pairs)
        for i, (l, r) in enumerate(pairs):
            nc.tensor.matmul(ps, self.cast(l, dt), self.cast(r, dt),
                             start=(i == 0), stop=(i == n - 1))

    def wslice(self, wt, name, kt, mo, mw=128):
        g, noff, kts, M, K = WNAMES[name]
        rows = min(128, K - kt * 128)
        base = noff + kt * M + mo
        return wt[0:rows, base:base + mw]

    def bap(self, name, mt=0):
        return self.bb_sb[:, BOFF[name] + mt: BOFF[name] + mt + 1]

    def load_w(self, group, tag="w"):
        nc = self.nc
        goff, gcols = GOFF[group]
        wt = self.wpool.tile([128, gcols], F32, tag=tag, name=f"w_{group}",
                             padded_shape=[128, 6144] if tag == "w" else None)
        nc.sync.dma_start(out=wt, in_=self.wb[:, goff:goff + gcols])
        return wt

    # ---------------- build ----------------
    def build(self):
        nc = self.nc
        with tile.TileContext(nc) as tc:
            with contextlib.ExitStack() as ES:
                self.tc = tc
                pers = ES.enter_context(tc.tile_pool(name="pers", bufs=1))
                self.wpool = ES.enter_context(tc.tile_pool(name="wpool", bufs=2))
                self.act = ES.enter_context(tc.tile_pool(name="act", bufs=1))
                self.pers = pers

                self.bb_sb = pers.tile([128, NBIAS], F32, name="bb_sb")
                nc.sync.dma_start(out=self.bb_sb, in_=self.bbp[:])
                self.c2_sb = pers.tile([1, 1024], F32, name="c2_sb")
                nc.sync.dma_start(out=self.c2_sb, in_=self.c2p[:])
                self.ones_col = pers.tile([128, 1], F32, name="ones_col")
                nc.vector.memset(self.ones_col, 1.0)
                self.ones_row = pers.tile([1, 128], F32, name="ones_row")
                nc.vector.memset(self.ones_row, 1.0)
                self.ident = pers.tile([128, 128], F32, name="ident")
                make_identity(nc, self.ident)
                self.eps_sb = pers.tile([1, 1], F32, name="eps_sb")
                nc.vector.memset(self.eps_sb, 1e-5)
                self.pos_sb = self.load_w("pers", tag="pos")

                with nc.named_scope("enc_conv"):
                    x2 = self.encoder_convs()
                with nc.named_scope("enc_tf"):
                    z = self.enc_transformer(x2)
                with nc.named_scope("vq"):
                    q = self.vq(z)
                with nc.named_scope("dec_tf"):
                    y = self.dec_transformer(q)
                with nc.named_scope("dec_conv"):
                    self.decoder_convs(y)

        nc.finalize()
        return nc

    # -------------------------------------------------- encoder convs
    def encoder_convs(self):
        nc, tc = self.nc, self.tc
        dt = DT_ENC
        with contextlib.ExitStack() as ES:
            s1 = ES.enter_context(tc.tile_pool(name="s1", bufs=1))
            pp = ES.enter_context(tc.tile_pool(name="pp_enc", bufs=1, space="PSUM"))
            h1p = [s1.tile([128, NSAMP * 324], F32, name=f"h1p{mt}") for mt in range(2)]
            r1 = [s1.tile([128, NSAMP * 256], F32, name=f"r1_{mt}") for mt in range(2)]
            x1p = [s1.tile([128, NSAMP * 324], F32, name=f"x1p{mt}") for mt in range(2)]
            for mt in range(2):
                nc.gpsimd.memset(h1p[mt], 0.0)
                nc.gpsimd.memset(x1p[mt], 0.0)

            with contextlib.ExitStack() as ES2:
                emb = ES2.enter_context(tc.tile_pool(name="emb", bufs=1))
                iota16 = emb.tile([16, 1], I32, name="iota16")
                nc.gpsimd.iota(iota16, pattern=[[0, 1]], base=0, channel_multiplier=1)
                iota16f = emb.tile([16, 1], F32, name="iota16f")
                nc.vector.tensor_copy(out=iota16f, in_=iota16)
                oh = emb.tile([16, NSAMP * 1156], F32, name="oh")
                nc.gpsimd.memset(oh, 0.0)
                for s in range(NSAMP):
                    tokb = emb.tile([16, 1024], I32, tag="tokb", bufs=2, name="tokb")
                    nc.sync.dma_start(out=tokb, in_=bass.AP(tensor=self.tokp, offset=s * 1024,
                                                            ap=[[0, 16], [1, 1024]]))
                    tokf = emb.tile([16, 1024], F32, tag="tokf", bufs=2, name="tokf")
                    nc.vector.tensor_copy(out=tokf, in_=tokb)
                    oh_int = vap(oh, s * 1156 + 35, [[34, 32], [1, 32]])
                    nc.vector.tensor_scalar(out=oh_int,
                                            in0=tokf.rearrange("p (y x) -> p y x", y=32),
                                            scalar1=iota16f, scalar2=None, op0=ALU.is_equal)

                w1a = self.load_w("s1a")
                for mt in range(2):
                    for nch in range(4):
                        s0 = 2 * nch
                        ps = pp.tile([128, 512], F32, tag="mm", bufs=6, name="ps_c1")
                        pairs = []
                        for t, (ky, kx) in enumerate(TAPS9):
                            rhs = vap(oh, s0 * 1156 + ky * 34 + kx, [[1156, 2], [68, 16], [2, 16]])
                            pairs.append((self.wslice(w1a, f"e1c1_t{t}", 0, mt * 128), rhs))
                        self.mm(ps, pairs, dt)
                        dst = vap(h1p[mt], s0 * 324 + 19, [[324, 2], [18, 16], [1, 16]])
                        nc.scalar.activation(dst, ps, AF.Relu, bias=self.bap("e1c1_b", mt))
                        ps2 = pp.tile([128, 512], F32, tag="mm", bufs=6, name="ps_sc")
                        rhs = vap(oh, s0 * 1156 + 35, [[1156, 2], [68, 16], [2, 16]])
                        self.mm(ps2, [(self.wslice(w1a, "e1sc", 0, mt * 128), rhs)], dt)
                        nc.scalar.activation(r1[mt][:, s0 * 256:s0 * 256 + 512], ps2,
                                             AF.Identity, bias=self.bap("e1sc_b", mt))

                w1b = self.load_w("s1b")
                for mt in range(2):
                    for nch in range(4):
                        s0 = 2 * nch
                        ps = pp.tile([128, 512], F32, tag="mm", bufs=6, name="ps_c2")
                        pairs = []
                        for t, (ky, kx) in enumerate(TAPS9):
                            for kt in range(2):
                                rhs = vap(h1p[kt], s0 * 324 + ky * 18 + kx, [[324, 2], [18, 16], [1, 16]])
                                pairs.append((self.wslice(w1b, f"e1c2_t{t}", kt, mt * 128), rhs))
                        self.mm(ps, pairs, dt)
                        tmp = s1.tile([128, 512], F32, tag="tmp", bufs=2, name="c2tmp")
                        nc.scalar.activation(tmp, ps, AF.Relu, bias=self.bap("e1c2_b", mt))
                        dst = vap(x1p[mt], s0 * 324 + 19, [[324, 2], [18, 16], [1, 16]])
                        rsl = vap(r1[mt], s0 * 256, [[256, 2], [16, 16], [1, 16]])
                        nc.vector.tensor_tensor(out=dst,
                                                in0=tmp.rearrange("p (s y x) -> p s y x", s=2, y=16),
                                                in1=rsl, op=ALU.add)

            with contextlib.ExitStack() as ES3:
                s1b = ES3.enter_context(tc.tile_pool(name="s1b", bufs=1))
                h2p = [s1b.tile([128, NSAMP * 100], F32, name=f"h2p{mt}") for mt in range(2)]
                for mt in range(2):
                    nc.gpsimd.memset(h2p[mt], 0.0)
                w2a = self.load_w("s2a")
                for mt in range(2):
                    ps = pp.tile([128, 512], F32, tag="mm", bufs=6, name="ps_b2c1")
                    pairs = []
                    for t, (ky, kx) in enumerate(TAPS9):
                        for kt in range(2):
                            rhs = vap(x1p[kt], ky * 18 + kx, [[324, NSAMP], [36, 8], [2, 8]])
                            pairs.append((self.wslice(w2a, f"e2c1_t{t}", kt, mt * 128), rhs))
                    self.mm(ps, pairs, dt)
                    dst = vap(h2p[mt], 11, [[100, NSAMP], [10, 8], [1, 8]])
                    nc.scalar.activation(dst, ps, AF.Relu, bias=self.bap("e2c1_b", mt))
                r2 = [s1b.tile([128, 512], F32, name=f"r2_{mt}") for mt in range(2)]
                for mt in range(2):
                    ps = pp.tile([128, 512], F32, tag="mm", bufs=6, name="ps_b2sc")
                    pairs = []
                    for kt in range(2):
                        rhs = vap(x1p[kt], 19, [[324, NSAMP], [36, 8], [2, 8]])
                        pairs.append((self.wslice(w2a, "e2sc", kt, mt * 128), rhs))
                    self.mm(ps, pairs, dt)
                    nc.scalar.activation(r2[mt], ps, AF.Identity, bias=self.bap("e2sc_b", mt))
                w2b = self.load_w("s2b")
                x2 = [self.act.tile([128, 512], F32, tag="x", bufs=4, name=f"x2_{mt}") for mt in range(2)]
                for mt in range(2):
                    ps = pp.tile([128, 512], F32, tag="mm", bufs=6, name="ps_b2c2")
                    pairs = []
                    for t, (ky, kx) in enumerate(TAPS9):
                        for kt in range(2):
                            rhs = vap(h2p[kt], ky * 10 + kx, [[100, NSAMP], [10, 8], [1, 8]])
                            pairs.append((self.wslice(w2b, f"e2c2_t{t}", kt, mt * 128), rhs))
                    self.mm(ps, pairs, dt)
                    tmp = s1b.tile([128, 512], F32, tag="tmpb", bufs=2, name="b2tmp")
                    nc.scalar.activation(tmp, ps, AF.Relu, bias=self.bap("e2c2_b", mt))
                    nc.vector.tensor_tensor(out=x2[mt], in0=tmp, in1=r2[mt], op=ALU.add)
        return x2

    # -------------------------------------------------- layernorm
    def ln(self, x2, pool, pp, gname=None, bname=None):
        nc = self.nc
        dt = DT_STAT
        sq = [pool.tile([128, 512], F32, tag="lnsq", bufs=2, name=f"sq{kt}") for kt in range(2)]
        for kt in range(2):
            nc.vector.tensor_tensor(out=sq[kt], in0=x2[kt], in1=x2[kt], op=ALU.mult)
        ps1 = pp.tile([1, 512], F32, tag="st", bufs=2, name="ps_s1")
        self.mm(ps1, [(self.ones_col, x2[0]), (self.ones_col, x2[1])], dt)
        ps2 = pp.tile([1, 512], F32, tag="st", bufs=2, name="ps_s2")
        self.mm(ps2, [(self.ones_col, sq[0]), (self.ones_col, sq[1])], dt)
        m = pool.tile([1, 512], F32, tag="lnrow", bufs=5, name="ln_m")
        nc.scalar.mul(out=m, in_=ps1, mul=1.0 / 256.0)
        msq = pool.tile([1, 512], F32, tag="lnrow", bufs=5, name="ln_msq")
        nc.vector.tensor_tensor(out=msq, in0=m, in1=m, op=ALU.mult)
        var = pool.tile([1, 512], F32, tag="lnrow", bufs=5, name="ln_var")
        nc.vector.scalar_tensor_tensor(out=var, in0=ps2, scalar=1.0 / 256.0, in1=msq,
                                       op0=ALU.mult, op1=ALU.subtract)
        sd = pool.tile([1, 512], F32, tag="lnrow", bufs=5, name="ln_sd")
        nc.scalar.activation(sd, var, AF.Sqrt, bias=self.eps_sb[0:1, 0:1])
        r = pool.tile([1, 512], F32, tag="lnrow", bufs=5, name="ln_r")
        nc.vector.reciprocal(r, sd)
        pm = pp.tile([128, 512], F32, tag="g", bufs=4, name="ps_bm")
        self.mm(pm, [(self.ones_row, m)], dt)
        pr = pp.tile([128, 512], F32, tag="g", bufs=4, name="ps_br")
        self.mm(pr, [(self.ones_row, r)], dt)
        out = []
        for kt in range(2):
            t1 = pool.tile([128, 512], F32, tag="lnt1", bufs=2, name=f"ln_t1_{kt}")
            nc.vector.tensor_tensor(out=t1, in0=x2[kt], in1=pm, op=ALU.subtract)
            o = pool.tile([128, 512], F32, tag="xn", bufs=4, name=f"ln_o{kt}")
            if gname is None:
                nc.vector.tensor_tensor(out=o, in0=t1, in1=pr, op=ALU.mult)
            else:
                t2 = pool.tile([128, 512], F32, tag="lnt2", bufs=2, name=f"ln_t2_{kt}")
                nc.vector.tensor_tensor(out=t2, in0=t1, in1=pr, op=ALU.mult)
                nc.scalar.activation(o, t2, AF.Identity,
                                     bias=self.bap(bname, kt), scale=self.bap(gname, kt))
            out.append(o)
        return out

    # -------------------------------------------------- transformer layer
    # psum tags in tf pools: g(4) + acc(2) + st(2) = 8 banks
    def tlayer(self, x2, g, pool, pp, dt):
        nc = self.nc
        wt = self.load_w(g)
        xn = self.ln(x2, pool, pp)
        qkv_sb = []
        for j in range(6):
            ps = pp.tile([128, 512], F32, tag="g", bufs=4, name=f"ps_qkv{j}")
            self.mm(ps, [(self.wslice(wt, f"{g}_qkv", kt, j * 128), xn[kt]) for kt in range(2)], dt)
            t = pool.tile([128, 512], F32, tag=f"qkv{j}", bufs=1, name=f"qkv{j}")
            nc.scalar.copy(out=t, in_=ps)
            qkv_sb.append(t)
        q_sb, k_sb, v_sb = qkv_sb[0:2], qkv_sb[2:4], qkv_sb[4:6]
        att_ps = [pp.tile([128, 512], F32, tag="acc", bufs=2, name=f"attps{mt}") for mt in range(2)]
        for h in range(NH):
            mt, ro = h // 2, (h % 2) * 64
            ps_sc = pp.tile([64, 512], F32, tag="g", bufs=4, name="ps_sc")
            for s in range(NSAMP):
                nc.tensor.matmul(ps_sc[0:64, s * 64:s * 64 + 64],
                                 self.cast(q_sb[mt][ro:ro + 64, s * 64:s * 64 + 64], dt),
                                 self.cast(k_sb[mt][ro:ro + 64, s * 64:s * 64 + 64], dt),
                                 start=True, stop=True)
            p_sb = pool.tile([64, 512], F32, tag="p", bufs=2, name="p_sb")
            nc.scalar.activation(p_sb, ps_sc[0:64, :], AF.Exp, scale=0.125)
            sums = pool.tile([64, 8], F32, tag="sums", bufs=2, name="sums")
            nc.vector.tensor_reduce(out=sums, in_=p_sb.rearrange("p (s n) -> p s n", n=64),
                                    axis=mybir.AxisListType.X, op=ALU.add)
            rec = pool.tile([64, 8], F32, tag="rec", bufs=2, name="rec")
            nc.vector.reciprocal(rec, sums)
            pn = pool.tile([64, 512], F32, tag="pn", bufs=2, name="pn")
            recb = bass.AP(tensor=rec.tensor, offset=rec.offset,
                           ap=[list(rec.ap[0]), [1, 8], [0, 64]])
            nc.vector.tensor_tensor(out=pn.rearrange("p (s n) -> p s n", n=64),
                                    in0=p_sb.rearrange("p (s n) -> p s n", n=64),
                                    in1=recb, op=ALU.mult)
            ps_pt = pp.tile([64, 512], F32, tag="g", bufs=4, name="ps_pt")
            for s in range(NSAMP):
                nc.tensor.transpose(ps_pt[0:64, s * 64:s * 64 + 64],
                                    pn[0:64, s * 64:s * 64 + 64], self.ident[0:64, 0:64])
            pt_sb = pool.tile([64, 512], F32, tag="pt", bufs=2, name="pt_sb")
            nc.scalar.copy(out=pt_sb, in_=ps_pt[0:64, :])
            ps_vt = pp.tile([64, 512], F32, tag="g", bufs=4, name="ps_vt")
            for s in range(NSAMP):
                nc.tensor.transpose(ps_vt[0:64, s * 64:s * 64 + 64],
                                    v_sb[mt][ro:ro + 64, s * 64:s * 64 + 64],
                                    self.ident[ro:ro + 64, ro:ro + 64])
            vt_sb = pool.tile([64, 512], F32, tag="vt", bufs=2, name="vt_sb")
            nc.scalar.copy(out=vt_sb, in_=ps_vt[0:64, :])
            for s in range(NSAMP):
                nc.tensor.matmul(att_ps[mt][ro:ro + 64, s * 64:s * 64 + 64],
                                 self.cast(vt_sb[0:64, s * 64:s * 64 + 64], dt),
                                 self.cast(pt_sb[0:64, s * 64:s * 64 + 64], dt),
                                 start=True, stop=True)
        att_sb = []
        for mt in range(2):
            t = pool.tile([128, 512], F32, tag=f"att_sb{mt}", bufs=1, name=f"att_sb{mt}")
            nc.scalar.copy(out=t, in_=att_ps[mt])
            att_sb.append(t)
        x_new = []
        for mt in range(2):
            ps = pp.tile([128, 512], F32, tag="g", bufs=4, name="ps_wo")
            self.mm(ps, [(self.wslice(wt, f"{g}_wo", kt, mt * 128), att_sb[kt]) for kt in range(2)], dt)
            xo = self.act.tile([128, 512], F32, tag="x", bufs=4, name=f"xa{mt}")
            nc.vector.tensor_tensor(out=xo, in0=x2[mt], in1=ps, op=ALU.add)
            x_new.append(xo)
        xn2 = self.ln(x_new, pool, pp)
        ps_o = [pp.tile([128, 512], F32, tag="acc", bufs=2, name=f"ps_ffn{mt}") for mt in range(2)]
        for j in range(8):
            ps = pp.tile([128, 512], F32, tag="g", bufs=4, name=f"ps_h{j}")
            self.mm(ps, [(self.wslice(wt, f"{g}_w1", kt, j * 128), xn2[kt]) for kt in range(2)], dt)
            hf = pool.tile([128, 512], F32, tag="hf", bufs=3, name=f"hf{j}")
            nc.scalar.activation(hf, ps, AF.Relu)
            for mt in range(2):
                nc.tensor.matmul(ps_o[mt], self.cast(self.wslice(wt, f"{g}_w2", j, mt * 128), dt),
                                 self.cast(hf, dt), start=(j == 0), stop=(j == 7))
        x_out = []
        for mt in range(2):
            xo = self.act.tile([128, 512], F32, tag="x", bufs=4, name=f"xf{mt}")
            nc.vector.tensor_tensor(out=xo, in0=x_new[mt], in1=ps_o[mt], op=ALU.add)
            x_out.append(xo)
        return x_out

    def enc_transformer(self, x2):
        nc, tc = self.nc, self.tc
        with contextlib.ExitStack() as ES:
            pool = ES.enter_context(tc.tile_pool(name="tf_e", bufs=1))
            pp = ES.enter_context(tc.tile_pool(name="pp_tfe", bufs=1, space="PSUM"))
            zc = self.ln(x2, pool, pp, gname="enc_ln_g", bname="enc_ln_b")
            z = []
            for kt in range(2):
                zz = self.act.tile([128, 512], F32, tag="x", bufs=4, name=f"z{kt}")
                nc.vector.tensor_tensor(out=zz, in0=zc[kt],
                                        in1=self.pos_sb[:, kt * 512:(kt + 1) * 512], op=ALU.add)
                z.append(zz)
            for i in range(NL):
                z = self.tlayer(z, f"te{i}", pool, pp, DT_ENC)
            zl = self.ln(z, pool, pp, gname="t_enc_ln_g", bname="t_enc_ln_b")
            zf = []
            for kt in range(2):
                zz = self.act.tile([128, 512], F32, tag="x", bufs=4, name=f"zf{kt}")
                nc.vector.tensor_copy(out=zz, in_=zl[kt])
                zf.append(zz)
            return zf

    # -------------------------------------------------- VQ
    def vq(self, z):
        nc, tc = self.nc, self.tc
        with contextlib.ExitStack() as ES:
            pool = ES.enter_context(tc.tile_pool(name="vqp", bufs=1))
            pp = ES.enter_context(tc.tile_pool(name="pp_vq", bufs=1, space="PSUM"))
            wt = self.load_w("vq")
            e_sb = []
            for t4 in range(4):
                ps = pp.tile([128, 1024], F32, tag="big", bufs=2, name="ps_vq")
                for half in range(2):
                    pairs = []
                    for kt in range(2):
                        pairs.append((z[kt][:, t4 * 128:(t4 + 1) * 128],
                                      self.wslice(wt, "vq_rhs", kt, half * 512, 512)))
                    pairs.append((self.ones_row, self.c2_sb[0:1, half * 512:(half + 1) * 512]))
                    self.mm(ps[:, half * 512:(half + 1) * 512], pairs, DT_VQ)
                s_sb = pool.tile([128, 1024], F32, tag="s", bufs=2, name="s_sb")
                nc.scalar.copy(out=s_sb, in_=ps)
                mx8 = pool.tile([128, 8], F32, tag="mx", bufs=2, name="mx8")
                nc.vector.max(mx8, s_sb)
                e = pool.tile([128, 1024], F32, tag=f"e{t4}", bufs=1, name=f"e{t4}")
                nc.vector.tensor_scalar(out=e, in0=s_sb, scalar1=mx8[:, 0:1], scalar2=None,
                                        op0=ALU.is_equal)
                e_sb.append(e)
            eT = []
            for ct in range(8):
                ps = pp.tile([128, 512], F32, tag="mm", bufs=2, name="ps_eT")
                for t4 in range(4):
                    nc.tensor.transpose(ps[:, t4 * 128:(t4 + 1) * 128],
                                        e_sb[t4][:, ct * 128:(ct + 1) * 128], self.ident)
                t = pool.tile([128, 512], F32, tag=f"eT{ct}", bufs=1, name=f"eT{ct}")
                nc.scalar.copy(out=t, in_=ps)
                eT.append(t)
            q = []
            for mt in range(2):
                ps = pp.tile([128, 512], F32, tag="mm", bufs=2, name="ps_q")
                self.mm(ps, [(self.wslice(wt, "cb", ct, mt * 128), eT[ct]) for ct in range(8)], DT_Q)
                qq = self.act.tile([128, 512], F32, tag="x", bufs=4, name=f"q{mt}")
                nc.scalar.copy(out=qq, in_=ps)
                q.append(qq)
            return q

    def dec_transformer(self, q):
        nc, tc = self.nc, self.tc
        with contextlib.ExitStack() as ES:
            pool = ES.enter_context(tc.tile_pool(name="tf_d", bufs=1))
            pp = ES.enter_context(tc.tile_pool(name="pp_tfd", bufs=1, space="PSUM"))
            y = q
            for i in range(NL):
                y = self.tlayer(y, f"td{i}", pool, pp, DT_DEC)
            return y

    # -------------------------------------------------- decoder convs
    def decoder_convs(self, y):
        nc, tc = self.nc, self.tc
        dt = DT_DEC
        classes = [
            ((0, 0), [(4, 0, 0)]),
            ((1, 0), [(1, 0, 0), (7, 1, 0)]),
            ((0, 1), [(3, 0, 0), (5, 0, 1)]),
            ((1, 1), [(0, 0, 0), (2, 0, 1), (6, 1, 0), (8, 1, 1)]),
        ]
        with contextlib.ExitStack() as ES:
            s8 = ES.enter_context(tc.tile_pool(name="s8", bufs=1))
            pp = ES.enter_context(tc.tile_pool(name="pp_dec", bufs=1, space="PSUM"))
            # long-lived within decoder: x1p, r2, out_sb
            x1p = [s8.tile([128, NSAMP * 324], F32, name=f"x1pd{mt}") for mt in range(2)]
            r2 = [s8.tile([128, 2048], F32, name=f"r2d{mt}") for mt in range(2)]
            out_sb = s8.tile([128, 64, 16], F32, name="out_sb")
            for mt in range(2):
                nc.gpsimd.memset(x1p[mt], 0.0)

            with contextlib.ExitStack() as ESa:
                d1 = ESa.enter_context(tc.tile_pool(name="d1p", bufs=1))
                yp = [d1.tile([128, NSAMP * 81], F32, name=f"yp{kt}") for kt in range(2)]
                for kt in range(2):
                    nc.gpsimd.memset(yp[kt], 0.0)
                    dst = vap(yp[kt], 0, [[81, NSAMP], [9, 8], [1, 8]])
                    nc.vector.tensor_copy(out=dst,
                                          in_=y[kt].rearrange("p (s a b) -> p s a b", a=8, b=8))
                h1p = [d1.tile([128, NSAMP * 324], F32, name=f"h1pd{mt}") for mt in range(2)]
                w = self.load_w("d1a")
                for mt in range(2):
                    nc.gpsimd.memset(h1p[mt], 0.0)
                    for (py, px), taps in classes:
                        ps = pp.tile([128, 512], F32, tag="mm", bufs=6, name="ps_d1c1")
                        pairs = []
                        for t, dy, dx in taps:
                            for kt in range(2):
                                rhs = vap(yp[kt], dy * 9 + dx, [[81, NSAMP], [9, 8], [1, 8]])
                                pairs.append((self.wslice(w, f"d1c1_t{t}", kt, mt * 128), rhs))
                        self.mm(ps, pairs, dt)
                        dst = vap(h1p[mt], (1 + py) * 18 + 1 + px, [[324, NSAMP], [36, 8], [2, 8]])
                        nc.scalar.activation(dst, ps, AF.Relu, bias=self.bap("d1c1_b", mt))
                r1 = [d1.tile([128, 512], F32, name=f"r1d{mt}") for mt in range(2)]
                for mt in range(2):
                    ps = pp.tile([128, 512], F32, tag="mm", bufs=6, name="ps_d1sc")
                    self.mm(ps, [(self.wslice(w, "d1sc", kt, mt * 128), y[kt]) for kt in range(2)], dt)
                    nc.scalar.copy(out=r1[mt], in_=ps)
                w = self.load_w("d1b")
                for mt in range(2):
                    for nch in range(4):
                        s0 = 2 * nch
                        ps = pp.tile([128, 512], F32, tag="mm", bufs=6, name="ps_d1c2")
                        pairs = []
                        for t, (ky, kx) in enumerate(TAPS9):
                            for kt in range(2):
                                rhs = vap(h1p[kt], s0 * 324 + ky * 18 + kx, [[324, 2], [18, 16], [1, 16]])
                                pairs.append((self.wslice(w, f"d1c2_t{t}", kt, mt * 128), rhs))
                        self.mm(ps, pairs, dt)
                        tmp = d1.tile([128, 512], F32, tag="tmp", bufs=2, name="d1tmp")
                        nc.scalar.activation(tmp, ps, AF.Relu, bias=self.bap("d1c2_b", mt))
                        dst = vap(x1p[mt], s0 * 324 + 19, [[324, 2], [18, 16], [1, 16]])
                        nc.vector.tensor_scalar(out=dst,
                                                in0=tmp.rearrange("p (s y x) -> p s y x", s=2, y=16),
                                                scalar1=self.bap("d1sc_b", mt), scalar2=None, op0=ALU.add)
                    ev = vap(x1p[mt], 19, [[324, NSAMP], [36, 8], [2, 8]])
                    nc.vector.tensor_tensor(out=ev, in0=ev,
                                            in1=r1[mt].rearrange("p (s a b) -> p s a b", a=8, b=8),
                                            op=ALU.add)

            with contextlib.ExitStack() as ESb:
                d2 = ESb.enter_context(tc.tile_pool(name="d2p", bufs=1))
                # d2 shortcut for all samples first (reads x1p only)
                w = self.load_w("d2a")
                for mt in range(2):
                    for nch in range(4):
                        s0 = 2 * nch
                        ps = pp.tile([128, 512], F32, tag="mm", bufs=6, name="ps_d2sc")
                        pairs = []
                        for kt in range(2):
                            rhs = vap(x1p[kt], s0 * 324 + 19, [[324, 2], [18, 16], [1, 16]])
                            pairs.append((self.wslice(w, "d2sc", kt, mt * 128), rhs))
                        self.mm(ps, pairs, dt)
                        nc.scalar.copy(out=r2[mt][:, s0 * 256:s0 * 256 + 512], in_=ps)
                w2b = self.load_w("d2b")
                # process sample-pairs: convT1 -> h2c chunk, then conv2+logits on it
                for p2 in range(4):
                    s0 = 2 * p2
                    h2c = [d2.tile([128, 2 * 1156], F32, tag=f"h2c{kt}", bufs=2, name=f"h2c{kt}")
                           for kt in range(2)]
                    for kt in range(2):
                        nc.gpsimd.memset(h2c[kt], 0.0)
                    for mt in range(2):
                        for (py, px), taps in classes:
                            ps = pp.tile([128, 512], F32, tag="mm", bufs=6, name="ps_d2c1")
                            pairs = []
                            for t, dy, dx in taps:
                                for kt in range(2):
                                    rhs = vap(x1p[kt], s0 * 324 + (1 + dy) * 18 + 1 + dx,
                                              [[324, 2], [18, 16], [1, 16]])
                                    pairs.append((self.wslice(w, f"d2c1_t{t}", kt, mt * 128), rhs))
                            self.mm(ps, pairs, dt)
                            dst = vap(h2c[mt], (1 + py) * 34 + 1 + px, [[1156, 2], [68, 16], [2, 16]])
                            nc.scalar.activation(dst, ps, AF.Relu, bias=self.bap("d2c1_b", mt))
                    for c in range(4):   # chunks within pair: (sample in pair, half)
                        sl, half = c // 2, c % 2
                        s = s0 + sl
                        r0 = 16 * half
                        xc = []
                        for mt in range(2):
                            ps = pp.tile([128, 512], F32, tag="mm", bufs=6, name="ps_d2c2")
                            pairs = []
                            for t, (ky, kx) in enumerate(TAPS9):
                                for kt in range(2):
                                    rhs = vap(h2c[kt], sl * 1156 + (ky + r0) * 34 + kx,
                                              [[34, 16], [1, 32]])
                                    pairs.append((self.wslice(w2b, f"d2c2_t{t}", kt, mt * 128), rhs))
                            self.mm(ps, pairs, dt)
                            tmp = d2.tile([128, 512], F32, tag="tmp2", bufs=2, name="d2tmp")
                            nc.scalar.activation(tmp, ps, AF.Relu, bias=self.bap("d2c2_b", mt))
                            xx = d2.tile([128, 512], F32, tag=f"xc{mt}", bufs=2, name=f"xc{mt}")
                            nc.vector.tensor_scalar(out=xx, in0=tmp, scalar1=self.bap("d2sc_b", mt),
                                                    scalar2=None, op0=ALU.add)
                            ev = vap(xx, 0, [[64, 8], [2, 16]])
                            rsl = vap(r2[mt], s * 256 + (r0 // 2) * 16, [[16, 8], [1, 16]])
                            nc.vector.tensor_tensor(out=ev, in0=ev, in1=rsl, op=ALU.add)
                            xc.append(xx)
                        ps_lg = pp.tile([16, 512], F32, tag="mm", bufs=6, name="ps_lg")
                        self.mm(ps_lg, [(self.wslice(w2b, "outw", kt, 0, 16), xc[kt])
                                        for kt in range(2)], DT_LG)
                        lg = d2.tile([16, 512], F32, tag="lg_sb", bufs=2, name="lg_sb")
                        nc.scalar.activation(lg, ps_lg[0:16, :], AF.Identity,
                                             bias=self.bb_sb[0:16, BOFF['out_b']:BOFF['out_b'] + 1])
                        cg = (p2 * 4 + c) * 4
                        for j in range(4):
                            ps_t = pp.tile([128, 16], F32, tag="mm", bufs=6, name="ps_lgt")
                            nc.tensor.transpose(ps_t, lg[0:16, j * 128:(j + 1) * 128],
                                                self.ident[0:16, 0:16])
                            nc.scalar.copy(out=out_sb[:, cg + j, :], in_=ps_t)
            dram_ap = bass.AP(tensor=self.outp, offset=0, ap=[[16, 128], [2048, 64], [1, 16]])
            nc.sync.dma_start(out=dram_ap, in_=out_sb)


# ---------------------------------------------------------------------------
# public entry
# ---------------------------------------------------------------------------

_NC_CACHE = {}


def _get_nc():
    if "nc" not in _NC_CACHE:
        _NC_CACHE["nc"] = KB().build()
    return _NC_CACHE["nc"]


def kernel(params=None, codebook=None, tokens=None, **kw):
    if params is None:
        params = kw['params']
    if codebook is None:
        codebook = kw['codebook']
    if tokens is None:
        tokens = kw['tokens']
    A = prep_arrays(params, codebook)
    wb, bb, c2 = pack_host(A)
    toks = np.ascontiguousarray(np.asarray(tokens).astype(np.int32).reshape(8, NSAMP * 1024))
    nc = _get_nc()
    in_maps = [{"wb": wb, "bb": bb, "c2": c2, "tok": toks[i][None, :]} for i in range(8)]
    res = run_bass_kernel_spmd(nc, in_maps, list(range(8)))
    out = np.concatenate([r["out"].reshape(NSAMP, 32, 32, NV) for r in res.results], axis=0)
    return out


# revision 14
# speedup vs baseline: 1.4907x; 1.4907x over previous
"""Trainium2 Bass kernel for nn_CTDAutoEncoder (VQ autoencoder forward).

Self-contained: kernel(**inputs) takes full inputs, shards batch 64 -> 8 cores
(8 samples each), runs one SPMD Bass kernel, gathers full output [64,32,32,16].

Device layout: activations channel-major [C(partitions), pix/tok(free)], C=256 ->
2 partition-tiles of 128. Convs = shifted-AP matmuls; BN folded into weights;
embed folded into enc conv1 (contraction over vocab=16); convT via parity classes;
LN stats via ones-matmul partition reduction; VQ argmin via DVE top-8 max of
s = 2*z.c - |c|^2 (z stationary -> scores token-major), onehot-matmul gather.
"""
import sys
sys.path.insert(0, '/opt/trn_rl_repo')
import contextlib
import numpy as np

import concourse.bass as bass
import concourse.tile as tile
from concourse import mybir, bacc
from concourse.bass_utils import run_bass_kernel_spmd
from concourse.masks import make_identity

F32 = mybir.dt.float32
F32R = mybir.dt.float32r
I32 = mybir.dt.int32
AF = mybir.ActivationFunctionType
ALU = mybir.AluOpType

C, NV, NH, NL = 256, 16, 4, 3
BNS = 1.0 / np.sqrt(1.0 + 1e-5)
NSAMP = 8
NTOK = NSAMP * 64

# per-stage matmul dtype: F32 (exact, 4cyc/row) or F32R (fast, reduced precision)
DT_ENC = F32
DT_VQ = F32
DT_Q = F32R
DT_DEC = F32R
DT_LG = F32R
DT_STAT_E = F32
DT_STAT_D = F32R

TAPS9 = [(ky, kx) for ky in range(3) for kx in range(3)]

# ---------------------------------------------------------------------------
# weight layout (shared host/builder)
# ---------------------------------------------------------------------------

def _wdefs():
    d = []
    for t in range(9):
        d.append(("s1a", f"e1c1_t{t}", NV, C))
    d.append(("s1a", "e1sc", NV, C))
    for t in range(9):
        d.append(("s1b", f"e1c2_t{t}", C, C))
    for t in range(9):
        d.append(("s2a", f"e2c1_t{t}", C, C))
    d.append(("s2a", "e2sc", C, C))
    for t in range(9):
        d.append(("s2b", f"e2c2_t{t}", C, C))
    for pre in ("te", "td"):
        for i in range(NL):
            g = f"{pre}{i}"
            d.append((g, f"{g}_qkv", C, 3 * C))
            d.append((g, f"{g}_wo", C, C))
            d.append((g, f"{g}_w1", C, 4 * C))
            d.append((g, f"{g}_w2", 4 * C, C))
    d.append(("vq", "vq_rhs", C, 1024))
    d.append(("vqcb", "cb", 1024, C))
    for t in range(9):
        d.append(("d1a", f"d1c1_t{t}", C, C))
    d.append(("d1a", "d1sc", C, C))
    for t in range(9):
        d.append(("d1b", f"d1c2_t{t}", C, C))
    for t in range(9):
        d.append(("d2a", f"d2c1_t{t}", C, C))
    d.append(("d2a", "d2sc", C, C))
    for t in range(9):
        d.append(("d2b", f"d2c2_t{t}", C, C))
    d.append(("d2b", "outw", C, NV))
    d.append(("pers", "pos", C, NTOK))
    return d


def _wlayout():
    groups, names, order = {}, {}, []
    for g, n, K, M in _wdefs():
        if g not in groups:
            groups[g] = 0
            order.append(g)
        kt = (K + 127) // 128
        names[n] = (g, groups[g], kt, M, K)
        groups[g] += kt * M
    goff, off = {}, 0
    for g in order:
        goff[g] = (off, groups[g])
        off += groups[g]
    return off, goff, names


WTOT, GOFF, WNAMES = _wlayout()
RGROUPS = {"td0", "td1", "td2", "d1a", "d1b", "d2a", "d2b", "vqcb"}


def _round_tf32(x):
    """round-to-nearest to 10 explicit mantissa bits (tf32-like) in place"""
    i = x.view(np.uint32)
    i += 0x1000
    i &= 0xFFFFE000
    return x

BIAS_NAMES = ["e1c1_b", "e1sc_b", "e1c2_b", "e2c1_b", "e2sc_b", "e2c2_b",
              "enc_ln_g", "enc_ln_b", "t_enc_ln_g", "t_enc_ln_b",
              "d1c1_b", "d1sc_b", "d1c2_b", "d2c1_b", "d2sc_b", "d2c2_b"]
BOFF = {n: 2 * i for i, n in enumerate(BIAS_NAMES)}
BOFF["out_b"] = 2 * len(BIAS_NAMES)
NBIAS = 2 * len(BIAS_NAMES) + 1


# ---------------------------------------------------------------------------
# host-side param prep
# ---------------------------------------------------------------------------

def _fold_bn(w, b, g, be, out_axis=0):
    s = (np.asarray(g, np.float64) * BNS)
    w = np.asarray(w, np.float64)
    shape = [1] * w.ndim
    shape[out_axis] = -1
    return (w * s.reshape(shape)).astype(np.float32), \
           (np.asarray(b, np.float64) * s + np.asarray(be, np.float64)).astype(np.float32)


def prep_arrays(params, codebook):
    A = {}
    E = np.asarray(params['embed'], np.float32)

    p0 = params['enc_blocks'][0]
    w1, b1 = _fold_bn(p0['w1'], p0['b1'], p0['g1'], p0['be1'])
    for t, (ky, kx) in enumerate(TAPS9):
        A[f'e1c1_t{t}'] = np.ascontiguousarray(E @ w1[:, :, ky, kx].T)
    A['e1c1_b'] = b1
    ws, bs = _fold_bn(p0['ws'], p0['bs'], p0['gs'], p0['bes'])
    A['e1sc'] = np.ascontiguousarray(E @ ws[:, :, 0, 0].T)
    A['e1sc_b'] = bs
    w2, b2 = _fold_bn(p0['w2'], p0['b2'], p0['g2'], p0['be2'])
    for t, (ky, kx) in enumerate(TAPS9):
        A[f'e1c2_t{t}'] = np.ascontiguousarray(w2[:, :, ky, kx].T)
    A['e1c2_b'] = b2

    p1 = params['enc_blocks'][1]
    w1, b1 = _fold_bn(p1['w1'], p1['b1'], p1['g1'], p1['be1'])
    for t, (ky, kx) in enumerate(TAPS9):
        A[f'e2c1_t{t}'] = np.ascontiguousarray(w1[:, :, ky, kx].T)
    A['e2c1_b'] = b1
    ws, bs = _fold_bn(p1['ws'], p1['bs'], p1['gs'], p1['bes'])
    A['e2sc'] = np.ascontiguousarray(ws[:, :, 0, 0].T)
    A['e2sc_b'] = bs
    w2, b2 = _fold_bn(p1['w2'], p1['b2'], p1['g2'], p1['be2'])
    for t, (ky, kx) in enumerate(TAPS9):
        A[f'e2c2_t{t}'] = np.ascontiguousarray(w2[:, :, ky, kx].T)
    A['e2c2_b'] = b2

    A['enc_ln_g'] = np.asarray(params['enc_ln_g'], np.float32)
    A['enc_ln_b'] = np.asarray(params['enc_ln_b'], np.float32)
    A['t_enc_ln_g'] = np.asarray(params['t_enc_ln_g'], np.float32)
    A['t_enc_ln_b'] = np.asarray(params['t_enc_ln_b'], np.float32)
    pos = np.asarray(params['pos'], np.float32)[0]
    A['pos'] = np.ascontiguousarray(np.tile(pos.T, (1, NSAMP)))

    for pre, layers in (('te', params['t_enc']), ('td', params['t_dec'])):
        for i, tp in enumerate(layers):
            g1 = np.asarray(tp['ln1'], np.float32)
            wqkv = np.asarray(tp['wqkv'], np.float32)
            A[f'{pre}{i}_qkv'] = np.ascontiguousarray((wqkv * g1[None, :]).T)
            A[f'{pre}{i}_wo'] = np.ascontiguousarray(np.asarray(tp['wo'], np.float32).T)
            g2 = np.asarray(tp['ln2'], np.float32)
            A[f'{pre}{i}_w1'] = np.ascontiguousarray((np.asarray(tp['w1'], np.float32) * g2[None, :]).T)
            A[f'{pre}{i}_w2'] = np.ascontiguousarray(np.asarray(tp['w2'], np.float32).T)

    cb = np.asarray(codebook, np.float32)
    A['vq_rhs'] = np.ascontiguousarray(2.0 * cb.T)
    A['vq_c2'] = -(cb.astype(np.float64) ** 2).sum(-1).astype(np.float32)[None]
    A['cb'] = cb

    for pre, pd in (('d1', params['dec_blocks'][0]), ('d2', params['dec_blocks'][1])):
        w1 = np.asarray(pd['w1'], np.float64)
        s = np.asarray(pd['g1'], np.float64) * BNS
        w1 = (w1 * s[None, :, None, None]).astype(np.float32)
        b1 = (np.asarray(pd['b1'], np.float64) * s + np.asarray(pd['be1'], np.float64)).astype(np.float32)
        for dy in range(3):
            for dx in range(3):
                A[f'{pre}c1_t{dy * 3 + dx}'] = np.ascontiguousarray(w1[:, :, 2 - dy, 2 - dx])
        A[f'{pre}c1_b'] = b1
        ws = np.asarray(pd['ws'], np.float64)
        ss = np.asarray(pd['gs'], np.float64) * BNS
        A[f'{pre}sc'] = np.ascontiguousarray((ws[:, :, 0, 0] * ss[None, :]).astype(np.float32))
        A[f'{pre}sc_b'] = (np.asarray(pd['bs'], np.float64) * ss + np.asarray(pd['bes'], np.float64)).astype(np.float32)
        w2, b2 = _fold_bn(pd['w2'], pd['b2'], pd['g2'], pd['be2'])
        for t, (ky, kx) in enumerate(TAPS9):
            A[f'{pre}c2_t{t}'] = np.ascontiguousarray(w2[:, :, ky, kx].T)
        A[f'{pre}c2_b'] = b2

    A['outw'] = np.ascontiguousarray(np.asarray(params['out_w'], np.float32).T)
    A['out_b'] = np.asarray(params['out_b'], np.float32)
    return A


def pack_host(A):
    wb = np.zeros((128, WTOT), np.float32)
    for n, (g, noff, kt, M, K) in WNAMES.items():
        off = GOFF[g][0] + noff
        arr = A[n]
        assert arr.shape == (K, M), (n, arr.shape, K, M)
        for k in range(kt):
            rows = min(128, K - k * 128)
            wb[0:rows, off + k * M: off + (k + 1) * M] = arr[k * 128:k * 128 + rows]
    for g in RGROUPS:
        off, cols = GOFF[g]
        _round_tf32(wb[:, off:off + cols])
    bb = np.zeros((128, NBIAS), np.float32)
    for n in BIAS_NAMES:
        arr = A[n]
        bb[:, BOFF[n]] = arr[0:128]
        bb[:, BOFF[n] + 1] = arr[128:256]
    bb[0:16, BOFF['out_b']] = A['out_b']
    return wb, bb, np.ascontiguousarray(A['vq_c2'])


# ---------------------------------------------------------------------------
# device kernel builder
# ---------------------------------------------------------------------------

def vap(t, off, dims):
    """manual free-dim AP view of a (possibly partition-sliced) tile AP"""
    return bass.AP(tensor=t.tensor, offset=t.offset + off,
                   ap=[list(t.ap[0])] + [list(d) for d in dims])


class KB:
    def __init__(self):
        self.nc = bacc.Bacc()
        nc = self.nc
        self.wb = nc.declare_dram_parameter("wb", [128, WTOT], F32, isOutput=False)
        self.bbp = nc.declare_dram_parameter("bb", [128, NBIAS], F32, isOutput=False)
        self.c2p = nc.declare_dram_parameter("c2", [1, 1024], F32, isOutput=False)
        self.tokp = nc.declare_dram_parameter("tok", [1, NSAMP * 1024], I32, isOutput=False)
        self.outp = nc.declare_dram_parameter("out", [NSAMP * 1024, NV], F32, isOutput=True)

    def cast(self, ap, dt):
        if dt is F32R and ap.dtype is not F32R:
            return ap.bitcast(F32R)
        return ap

    def mm(self, ps, pairs, dt):
        nc = self.nc
        n = len(pairs)
        for i, (l, r) in enumerate(pairs):
            nc.tensor.matmul(ps, self.cast(l, dt), self.cast(r, dt),
                             start=(i == 0), stop=(i == n - 1))

    def wslice(self, wt, name, kt, mo, mw=128):
        g, noff, kts, M, K = WNAMES[name]
        rows = min(128, K - kt * 128)
        base = noff + kt * M + mo
        return wt[0:rows, base:base + mw]

    def bap(self, name, mt=0):
        return self.bb_sb[:, BOFF[name] + mt: BOFF[name] + mt + 1]

    def load_w(self, group, tag="w"):
        nc = self.nc
        goff, gcols = GOFF[group]
        wdt = F32R if group in RGROUPS else F32
        wt = self.wpool.tile([128, gcols], wdt, tag=tag, name=f"w_{group}",
                             padded_shape=[128, 6144] if tag == "w" else None)
        src_ap = self.wb[:, goff:goff + gcols]
        if wdt is F32R:
            src_ap = src_ap.bitcast(F32R)
        nc.sync.dma_start(out=wt, in_=src_ap)
        return wt

    # ---------------- build ----------------
    def build(self):
        nc = self.nc
        with tile.TileContext(nc) as tc:
            with contextlib.ExitStack() as ES:
                self.tc = tc
                pers = ES.enter_context(tc.tile_pool(name="pers", bufs=1))
                self.wpool = ES.enter_context(tc.tile_pool(name="wpool", bufs=2))
                self.act = ES.enter_context(tc.tile_pool(name="act", bufs=1))
                self.pers = pers

                self.bb_sb = pers.tile([128, NBIAS], F32, name="bb_sb")
                nc.sync.dma_start(out=self.bb_sb, in_=self.bbp[:])
                self.c2_sb = pers.tile([1, 1024], F32, name="c2_sb")
                nc.sync.dma_start(out=self.c2_sb, in_=self.c2p[:])
                self.ones_col = pers.tile([128, 1], F32, name="ones_col")
                nc.vector.memset(self.ones_col, 1.0)
                self.ones_row = pers.tile([1, 128], F32, name="ones_row")
                nc.vector.memset(self.ones_row, 1.0)
                self.ident = pers.tile([128, 128], F32, name="ident")
                make_identity(nc, self.ident)
                self.eps_sb = pers.tile([1, 1], F32, name="eps_sb")
                nc.vector.memset(self.eps_sb, 1e-5)
                self.ones_col_r = pers.tile([128, 1], F32R, name="ones_col_r")
                nc.scalar.copy(out=self.ones_col_r, in_=self.ones_col)
                self.ones_row_r = pers.tile([1, 128], F32R, name="ones_row_r")
                nc.scalar.copy(out=self.ones_row_r, in_=self.ones_row)
                self.pos_sb = self.load_w("pers", tag="pos")

                with nc.named_scope("enc_conv"):
                    x2 = self.encoder_convs()
                with nc.named_scope("enc_tf"):
                    z = self.enc_transformer(x2)
                with nc.named_scope("vq"):
                    q = self.vq(z)
                with nc.named_scope("dec_tf"):
                    y = self.dec_transformer(q)
                with nc.named_scope("dec_conv"):
                    self.decoder_convs(y)

        nc.finalize()
        return nc

    # -------------------------------------------------- encoder convs
    def encoder_convs(self):
        nc, tc = self.nc, self.tc
        dt = DT_ENC
        with contextlib.ExitStack() as ES:
            s1 = ES.enter_context(tc.tile_pool(name="s1", bufs=1))
            pp = ES.enter_context(tc.tile_pool(name="pp_enc", bufs=1, space="PSUM"))
            h1p = [s1.tile([128, NSAMP * 324], F32, name=f"h1p{mt}") for mt in range(2)]
            r1 = [s1.tile([128, NSAMP * 256], F32, name=f"r1_{mt}") for mt in range(2)]
            x1p = [s1.tile([128, NSAMP * 324], F32, name=f"x1p{mt}") for mt in range(2)]
            for mt in range(2):
                nc.gpsimd.memset(h1p[mt], 0.0)
                nc.gpsimd.memset(x1p[mt], 0.0)

            with contextlib.ExitStack() as ES2:
                emb = ES2.enter_context(tc.tile_pool(name="emb", bufs=1))
                iota16 = emb.tile([16, 1], I32, name="iota16")
                nc.gpsimd.iota(iota16, pattern=[[0, 1]], base=0, channel_multiplier=1)
                iota16f = emb.tile([16, 1], F32, name="iota16f")
                nc.vector.tensor_copy(out=iota16f, in_=iota16)
                oh = emb.tile([16, NSAMP * 1156], F32, name="oh")
                nc.gpsimd.memset(oh, 0.0)
                for s in range(NSAMP):
                    tokb = emb.tile([16, 1024], I32, tag="tokb", bufs=2, name="tokb")
                    nc.sync.dma_start(out=tokb, in_=bass.AP(tensor=self.tokp, offset=s * 1024,
                                                            ap=[[0, 16], [1, 1024]]))
                    tokf = emb.tile([16, 1024], F32, tag="tokf", bufs=2, name="tokf")
                    nc.vector.tensor_copy(out=tokf, in_=tokb)
                    oh_int = vap(oh, s * 1156 + 35, [[34, 32], [1, 32]])
                    nc.vector.tensor_scalar(out=oh_int,
                                            in0=tokf.rearrange("p (y x) -> p y x", y=32),
                                            scalar1=iota16f, scalar2=None, op0=ALU.is_equal)

                w1a = self.load_w("s1a")
                for mt in range(2):
                    for nch in range(4):
                        s0 = 2 * nch
                        ps = pp.tile([128, 512], F32, tag="mm", bufs=6, name="ps_c1")
                        pairs = []
                        for t, (ky, kx) in enumerate(TAPS9):
                            rhs = vap(oh, s0 * 1156 + ky * 34 + kx, [[1156, 2], [68, 16], [2, 16]])
                            pairs.append((self.wslice(w1a, f"e1c1_t{t}", 0, mt * 128), rhs))
                        self.mm(ps, pairs, dt)
                        dst = vap(h1p[mt], s0 * 324 + 19, [[324, 2], [18, 16], [1, 16]])
                        nc.scalar.activation(dst, ps, AF.Relu, bias=self.bap("e1c1_b", mt))
                        ps2 = pp.tile([128, 512], F32, tag="mm", bufs=6, name="ps_sc")
                        rhs = vap(oh, s0 * 1156 + 35, [[1156, 2], [68, 16], [2, 16]])
                        self.mm(ps2, [(self.wslice(w1a, "e1sc", 0, mt * 128), rhs)], dt)
                        nc.scalar.activation(r1[mt][:, s0 * 256:s0 * 256 + 512], ps2,
                                             AF.Identity, bias=self.bap("e1sc_b", mt))

                w1b = self.load_w("s1b")
                for mt in range(2):
                    for nch in range(4):
                        s0 = 2 * nch
                        ps = pp.tile([128, 512], F32, tag="mm", bufs=6, name="ps_c2")
                        pairs = []
                        for t, (ky, kx) in enumerate(TAPS9):
                            for kt in range(2):
                                rhs = vap(h1p[kt], s0 * 324 + ky * 18 + kx, [[324, 2], [18, 16], [1, 16]])
                                pairs.append((self.wslice(w1b, f"e1c2_t{t}", kt, mt * 128), rhs))
                        self.mm(ps, pairs, dt)
                        tmp = s1.tile([128, 512], F32, tag="tmp", bufs=2, name="c2tmp")
                        nc.scalar.activation(tmp, ps, AF.Relu, bias=self.bap("e1c2_b", mt))
                        dst = vap(x1p[mt], s0 * 324 + 19, [[324, 2], [18, 16], [1, 16]])
                        rsl = vap(r1[mt], s0 * 256, [[256, 2], [16, 16], [1, 16]])
                        nc.vector.tensor_tensor(out=dst,
                                                in0=tmp.rearrange("p (s y x) -> p s y x", s=2, y=16),
                                                in1=rsl, op=ALU.add)

            with contextlib.ExitStack() as ES3:
                s1b = ES3.enter_context(tc.tile_pool(name="s1b", bufs=1))
                h2p = [s1b.tile([128, NSAMP * 100], F32, name=f"h2p{mt}") for mt in range(2)]
                for mt in range(2):
                    nc.gpsimd.memset(h2p[mt], 0.0)
                w2a = self.load_w("s2a")
                for mt in range(2):
                    ps = pp.tile([128, 512], F32, tag="mm", bufs=6, name="ps_b2c1")
                    pairs = []
                    for t, (ky, kx) in enumerate(TAPS9):
                        for kt in range(2):
                            rhs = vap(x1p[kt], ky * 18 + kx, [[324, NSAMP], [36, 8], [2, 8]])
                            pairs.append((self.wslice(w2a, f"e2c1_t{t}", kt, mt * 128), rhs))
                    self.mm(ps, pairs, dt)
                    dst = vap(h2p[mt], 11, [[100, NSAMP], [10, 8], [1, 8]])
                    nc.scalar.activation(dst, ps, AF.Relu, bias=self.bap("e2c1_b", mt))
                r2 = [s1b.tile([128, 512], F32, name=f"r2_{mt}") for mt in range(2)]
                for mt in range(2):
                    ps = pp.tile([128, 512], F32, tag="mm", bufs=6, name="ps_b2sc")
                    pairs = []
                    for kt in range(2):
                        rhs = vap(x1p[kt], 19, [[324, NSAMP], [36, 8], [2, 8]])
                        pairs.append((self.wslice(w2a, "e2sc", kt, mt * 128), rhs))
                    self.mm(ps, pairs, dt)
                    nc.scalar.activation(r2[mt], ps, AF.Identity, bias=self.bap("e2sc_b", mt))
                w2b = self.load_w("s2b")
                x2 = [self.act.tile([128, 512], F32, tag="x", bufs=4, name=f"x2_{mt}") for mt in range(2)]
                for mt in range(2):
                    ps = pp.tile([128, 512], F32, tag="mm", bufs=6, name="ps_b2c2")
                    pairs = []
                    for t, (ky, kx) in enumerate(TAPS9):
                        for kt in range(2):
                            rhs = vap(h2p[kt], ky * 10 + kx, [[100, NSAMP], [10, 8], [1, 8]])
                            pairs.append((self.wslice(w2b, f"e2c2_t{t}", kt, mt * 128), rhs))
                    self.mm(ps, pairs, dt)
                    tmp = s1b.tile([128, 512], F32, tag="tmpb", bufs=2, name="b2tmp")
                    nc.scalar.activation(tmp, ps, AF.Relu, bias=self.bap("e2c2_b", mt))
                    nc.vector.tensor_tensor(out=x2[mt], in0=tmp, in1=r2[mt], op=ALU.add)
        return x2

    # -------------------------------------------------- layernorm
    def ln(self, x2, pool, pp, gname=None, bname=None, dt=F32):
        nc = self.nc
        rr = dt is F32R
        oc = self.ones_col_r if rr else self.ones_col
        orow = self.ones_row_r if rr else self.ones_row

        def w(ap):  # producer out view for f32r-consumed tensors
            return ap.bitcast(F32R) if rr else ap

        sq = [pool.tile([128, 512], F32, tag="lnsq", bufs=2, name=f"sq{kt}") for kt in range(2)]
        for kt in range(2):
            nc.vector.tensor_tensor(out=w(sq[kt]), in0=x2[kt], in1=x2[kt], op=ALU.mult)
        ps1 = pp.tile([1, 512], F32, tag="st", bufs=2, name="ps_s1")
        self.mm(ps1, [(oc, x2[0]), (oc, x2[1])], dt)
        ps2 = pp.tile([1, 512], F32, tag="st", bufs=2, name="ps_s2")
        self.mm(ps2, [(oc, sq[0]), (oc, sq[1])], dt)
        m = pool.tile([1, 512], F32, tag="lnrow", bufs=5, name="ln_m")
        nc.scalar.mul(out=w(m), in_=ps1, mul=1.0 / 256.0)
        msq = pool.tile([1, 512], F32, tag="lnrow", bufs=5, name="ln_msq")
        nc.vector.tensor_tensor(out=msq, in0=m, in1=m, op=ALU.mult)
        var = pool.tile([1, 512], F32, tag="lnrow", bufs=5, name="ln_var")
        nc.vector.scalar_tensor_tensor(out=var, in0=ps2, scalar=1.0 / 256.0, in1=msq,
                                       op0=ALU.mult, op1=ALU.subtract)
        sd = pool.tile([1, 512], F32, tag="lnrow", bufs=5, name="ln_sd")
        nc.scalar.activation(sd, var, AF.Sqrt, bias=self.eps_sb[0:1, 0:1])
        r = pool.tile([1, 512], F32, tag="lnrow", bufs=5, name="ln_r")
        nc.vector.reciprocal(r, sd)
        if rr:
            rcast = pool.tile([1, 512], F32, tag="lnrow", bufs=5, name="ln_rr")
            nc.scalar.copy(out=rcast.bitcast(F32R), in_=r)
            r = rcast
        pm = pp.tile([128, 512], F32, tag="g", bufs=4, name="ps_bm")
        self.mm(pm, [(orow, m)], dt)
        pr = pp.tile([128, 512], F32, tag="g", bufs=4, name="ps_br")
        self.mm(pr, [(orow, r)], dt)
        out = []
        for kt in range(2):
            t1 = pool.tile([128, 512], F32, tag="lnt1", bufs=2, name=f"ln_t1_{kt}")
            nc.vector.tensor_tensor(out=t1, in0=x2[kt], in1=pm, op=ALU.subtract)
            o = pool.tile([128, 512], F32, tag="xn", bufs=4, name=f"ln_o{kt}")
            if gname is None:
                nc.vector.tensor_tensor(out=w(o), in0=t1, in1=pr, op=ALU.mult)
            else:
                t2 = pool.tile([128, 512], F32, tag="lnt2", bufs=2, name=f"ln_t2_{kt}")
                nc.vector.tensor_tensor(out=t2, in0=t1, in1=pr, op=ALU.mult)
                nc.scalar.activation(o, t2, AF.Identity,
                                     bias=self.bap(bname, kt), scale=self.bap(gname, kt))
            out.append(o)
        return out

    # -------------------------------------------------- transformer layer
    # psum tags in tf pools: g(4) + acc(2) + st(2) = 8 banks
    def tlayer(self, x2, g, pool, pp, dt):
        nc = self.nc
        rr = dt is F32R

        def w(ap):
            return ap.bitcast(F32R) if rr else ap

        wt = self.load_w(g)
        xn = self.ln(x2, pool, pp, dt=dt if dt is F32R else F32)
        qkv_sb = []
        for j in range(6):
            ps = pp.tile([128, 512], F32, tag="g", bufs=4, name=f"ps_qkv{j}")
            self.mm(ps, [(self.wslice(wt, f"{g}_qkv", kt, j * 128), xn[kt]) for kt in range(2)], dt)
            t = pool.tile([128, 512], F32, tag=f"qkv{j}", bufs=1, name=f"qkv{j}")
            nc.scalar.copy(out=t if j >= 4 else w(t), in_=ps)
            qkv_sb.append(t)
        q_sb, k_sb, v_sb = qkv_sb[0:2], qkv_sb[2:4], qkv_sb[4:6]
        att_sb = [pool.tile([128, 512], F32, tag=f"att_sb{mt}", bufs=1, name=f"att_sb{mt}")
                  for mt in range(2)]
        for h in range(NH):
            mt, ro = h // 2, (h % 2) * 64
            ps_sc = pp.tile([64, 512], F32, tag="g", bufs=4, name="ps_sc")
            for s in range(NSAMP):
                nc.tensor.matmul(ps_sc[0:64, s * 64:s * 64 + 64],
                                 self.cast(q_sb[mt][ro:ro + 64, s * 64:s * 64 + 64], dt),
                                 self.cast(k_sb[mt][ro:ro + 64, s * 64:s * 64 + 64], dt),
                                 start=True, stop=True)
            p_sb = pool.tile([64, 512], F32, tag="p", bufs=2, name="p_sb")
            nc.scalar.activation(p_sb, ps_sc[0:64, :], AF.Exp, scale=0.125)
            sums = pool.tile([64, 8], F32, tag="sums", bufs=2, name="sums")
            nc.vector.tensor_reduce(out=sums, in_=p_sb.rearrange("p (s n) -> p s n", n=64),
                                    axis=mybir.AxisListType.X, op=ALU.add)
            rec = pool.tile([64, 8], F32, tag="rec", bufs=2, name="rec")
            nc.vector.reciprocal(rec, sums)
            pn = pool.tile([64, 512], F32, tag="pn", bufs=2, name="pn")
            recb = bass.AP(tensor=rec.tensor, offset=rec.offset,
                           ap=[list(rec.ap[0]), [1, 8], [0, 64]])
            nc.vector.tensor_tensor(out=pn.rearrange("p (s n) -> p s n", n=64),
                                    in0=p_sb.rearrange("p (s n) -> p s n", n=64),
                                    in1=recb, op=ALU.mult)
            ps_pt = pp.tile([64, 512], F32, tag="g", bufs=4, name="ps_pt")
            for s in range(NSAMP):
                nc.tensor.transpose(ps_pt[0:64, s * 64:s * 64 + 64],
                                    pn[0:64, s * 64:s * 64 + 64], self.ident[0:64, 0:64])
            pt_sb = pool.tile([64, 512], F32, tag="pt", bufs=2, name="pt_sb")
            nc.scalar.copy(out=w(pt_sb), in_=ps_pt[0:64, :])
            ps_vt = pp.tile([64, 512], F32, tag="g", bufs=4, name="ps_vt")
            for s in range(NSAMP):
                nc.tensor.transpose(ps_vt[0:64, s * 64:s * 64 + 64],
                                    v_sb[mt][ro:ro + 64, s * 64:s * 64 + 64],
                                    self.ident[ro:ro + 64, ro:ro + 64])
            vt_sb = pool.tile([64, 512], F32, tag="vt", bufs=2, name="vt_sb")
            nc.scalar.copy(out=w(vt_sb), in_=ps_vt[0:64, :])
            ao_ps = pp.tile([64, 512], F32, tag="g", bufs=4, name="ao_ps")
            for s in range(NSAMP):
                nc.tensor.matmul(ao_ps[0:64, s * 64:s * 64 + 64],
                                 self.cast(vt_sb[0:64, s * 64:s * 64 + 64], dt),
                                 self.cast(pt_sb[0:64, s * 64:s * 64 + 64], dt),
                                 start=True, stop=True)
            nc.scalar.copy(out=w(att_sb[mt][ro:ro + 64, :]), in_=ao_ps[0:64, :])
        x_new = []
        for mt in range(2):
            ps = pp.tile([128, 512], F32, tag="g", bufs=4, name="ps_wo")
            self.mm(ps, [(self.wslice(wt, f"{g}_wo", kt, mt * 128), att_sb[kt]) for kt in range(2)], dt)
            xo = self.act.tile([128, 512], F32, tag="x", bufs=4, name=f"xa{mt}")
            nc.vector.tensor_tensor(out=w(xo), in0=x2[mt], in1=ps, op=ALU.add)
            x_new.append(xo)
        xn2 = self.ln(x_new, pool, pp, dt=dt if dt is F32R else F32)
        ps_o = [pp.tile([128, 512], F32, tag="acc", bufs=2, name=f"ps_ffn{mt}") for mt in range(2)]
        for j in range(8):
            ps = pp.tile([128, 512], F32, tag="g", bufs=4, name=f"ps_h{j}")
            self.mm(ps, [(self.wslice(wt, f"{g}_w1", kt, j * 128), xn2[kt]) for kt in range(2)], dt)
            hf = pool.tile([128, 512], F32, tag="hf", bufs=3, name=f"hf{j}")
            nc.scalar.activation(w(hf), ps, AF.Relu)
            for mt in range(2):
                nc.tensor.matmul(ps_o[mt], self.cast(self.wslice(wt, f"{g}_w2", j, mt * 128), dt),
                                 self.cast(hf, dt), start=(j == 0), stop=(j == 7))
        x_out = []
        for mt in range(2):
            xo = self.act.tile([128, 512], F32, tag="x", bufs=4, name=f"xf{mt}")
            nc.vector.tensor_tensor(out=w(xo), in0=x_new[mt], in1=ps_o[mt], op=ALU.add)
            x_out.append(xo)
        return x_out

    def enc_transformer(self, x2):
        nc, tc = self.nc, self.tc
        with contextlib.ExitStack() as ES:
            pool = ES.enter_context(tc.tile_pool(name="tf_e", bufs=1))
            pp = ES.enter_context(tc.tile_pool(name="pp_tfe", bufs=1, space="PSUM"))
            zc = self.ln(x2, pool, pp, gname="enc_ln_g", bname="enc_ln_b")
            z = []
            for kt in range(2):
                zz = self.act.tile([128, 512], F32, tag="x", bufs=4, name=f"z{kt}")
                nc.vector.tensor_tensor(out=zz, in0=zc[kt],
                                        in1=self.pos_sb[:, kt * 512:(kt + 1) * 512], op=ALU.add)
                z.append(zz)
            for i in range(NL):
                z = self.tlayer(z, f"te{i}", pool, pp, DT_ENC)
            zl = self.ln(z, pool, pp, gname="t_enc_ln_g", bname="t_enc_ln_b")
            zf = []
            for kt in range(2):
                zz = self.act.tile([128, 512], F32, tag="x", bufs=4, name=f"zf{kt}")
                nc.vector.tensor_copy(out=zz, in_=zl[kt])
                zf.append(zz)
            return zf

    # -------------------------------------------------- VQ
    def vq(self, z):
        nc, tc = self.nc, self.tc
        with contextlib.ExitStack() as ES:
            pool = ES.enter_context(tc.tile_pool(name="vqp", bufs=1))
            pp = ES.enter_context(tc.tile_pool(name="pp_vq", bufs=1, space="PSUM"))
            wt = self.load_w("vq")
            wtc = self.load_w("vqcb")
            e_sb = []
            for t4 in range(4):
                ps = pp.tile([128, 1024], F32, tag="big", bufs=2, name="ps_vq")
                for half in range(2):
                    pairs = []
                    for kt in range(2):
                        pairs.append((z[kt][:, t4 * 128:(t4 + 1) * 128],
                                      self.wslice(wt, "vq_rhs", kt, half * 512, 512)))
                    pairs.append((self.ones_row, self.c2_sb[0:1, half * 512:(half + 1) * 512]))
                    self.mm(ps[:, half * 512:(half + 1) * 512], pairs, DT_VQ)
                s_sb = pool.tile([128, 1024], F32, tag="s", bufs=2, name="s_sb")
                nc.scalar.copy(out=s_sb, in_=ps)
                mx8 = pool.tile([128, 8], F32, tag="mx", bufs=2, name="mx8")
                nc.vector.max(mx8, s_sb)
                e = pool.tile([128, 1024], F32, tag=f"e{t4}", bufs=1, name=f"e{t4}")
                nc.vector.tensor_scalar(out=e, in0=s_sb, scalar1=mx8[:, 0:1], scalar2=None,
                                        op0=ALU.is_equal)
                e_sb.append(e)
            eT = []
            for ct in range(8):
                ps = pp.tile([128, 512], F32, tag="mm", bufs=2, name="ps_eT")
                for t4 in range(4):
                    nc.tensor.transpose(ps[:, t4 * 128:(t4 + 1) * 128],
                                        e_sb[t4][:, ct * 128:(ct + 1) * 128], self.ident)
                t = pool.tile([128, 512], F32, tag=f"eT{ct}", bufs=1, name=f"eT{ct}")
                nc.scalar.copy(out=t.bitcast(F32R) if DT_Q is F32R else t, in_=ps)
                eT.append(t)
            q = []
            for mt in range(2):
                ps = pp.tile([128, 512], F32, tag="mm", bufs=2, name="ps_q")
                self.mm(ps, [(self.wslice(wtc, "cb", ct, mt * 128), eT[ct]) for ct in range(8)], DT_Q)
                qq = self.act.tile([128, 512], F32, tag="x", bufs=4, name=f"q{mt}")
                nc.scalar.copy(out=qq.bitcast(F32R) if DT_DEC is F32R else qq, in_=ps)
                q.append(qq)
            return q

    def dec_transformer(self, q):
        nc, tc = self.nc, self.tc
        with contextlib.ExitStack() as ES:
            pool = ES.enter_context(tc.tile_pool(name="tf_d", bufs=1))
            pp = ES.enter_context(tc.tile_pool(name="pp_tfd", bufs=1, space="PSUM"))
            y = q
            for i in range(NL):
                y = self.tlayer(y, f"td{i}", pool, pp, DT_DEC)
            return y

    # -------------------------------------------------- decoder convs
    def decoder_convs(self, y):
        nc, tc = self.nc, self.tc
        dt = DT_DEC
        classes = [
            ((0, 0), [(4, 0, 0)]),
            ((1, 0), [(1, 0, 0), (7, 1, 0)]),
            ((0, 1), [(3, 0, 0), (5, 0, 1)]),
            ((1, 1), [(0, 0, 0), (2, 0, 1), (6, 1, 0), (8, 1, 1)]),
        ]
        rr = dt is F32R

        def w(ap):
            return ap.bitcast(F32R) if rr else ap

        with contextlib.ExitStack() as ES:
            s8 = ES.enter_context(tc.tile_pool(name="s8", bufs=1))
            pp = ES.enter_context(tc.tile_pool(name="pp_dec", bufs=1, space="PSUM"))
            # long-lived within decoder: x1p, r2, out_sb
            x1p = [s8.tile([128, NSAMP * 324], F32, name=f"x1pd{mt}") for mt in range(2)]
            r2 = [s8.tile([128, 2048], F32, name=f"r2d{mt}") for mt in range(2)]
            out_sb = s8.tile([128, 64, 16], F32, name="out_sb")
            for mt in range(2):
                nc.gpsimd.memset(x1p[mt], 0.0)

            with contextlib.ExitStack() as ESa:
                d1 = ESa.enter_context(tc.tile_pool(name="d1p", bufs=1))
                yp = [d1.tile([128, NSAMP * 81], F32, name=f"yp{kt}") for kt in range(2)]
                for kt in range(2):
                    nc.gpsimd.memset(yp[kt], 0.0)
                    dst = vap(yp[kt], 0, [[81, NSAMP], [9, 8], [1, 8]])
                    nc.vector.tensor_copy(out=w(dst),
                                          in_=y[kt].rearrange("p (s a b) -> p s a b", a=8, b=8))
                h1p = [d1.tile([128, NSAMP * 324], F32, name=f"h1pd{mt}") for mt in range(2)]
                wd = self.load_w("d1a")
                for mt in range(2):
                    nc.gpsimd.memset(h1p[mt], 0.0)
                    for (py, px), taps in classes:
                        ps = pp.tile([128, 512], F32, tag="mm", bufs=6, name="ps_d1c1")
                        pairs = []
                        for t, dy, dx in taps:
                            for kt in range(2):
                                rhs = vap(yp[kt], dy * 9 + dx, [[81, NSAMP], [9, 8], [1, 8]])
                                pairs.append((self.wslice(wd, f"d1c1_t{t}", kt, mt * 128), rhs))
                        self.mm(ps, pairs, dt)
                        dst = vap(h1p[mt], (1 + py) * 18 + 1 + px, [[324, NSAMP], [36, 8], [2, 8]])
                        nc.scalar.activation(w(dst), ps, AF.Relu, bias=self.bap("d1c1_b", mt))
                r1 = [d1.tile([128, 512], F32, name=f"r1d{mt}") for mt in range(2)]
                for mt in range(2):
                    ps = pp.tile([128, 512], F32, tag="mm", bufs=6, name="ps_d1sc")
                    self.mm(ps, [(self.wslice(wd, "d1sc", kt, mt * 128), y[kt]) for kt in range(2)], dt)
                    nc.scalar.copy(out=r1[mt], in_=ps)
                wd = self.load_w("d1b")
                for mt in range(2):
                    for nch in range(4):
                        s0 = 2 * nch
                        ps = pp.tile([128, 512], F32, tag="mm", bufs=6, name="ps_d1c2")
                        pairs = []
                        for t, (ky, kx) in enumerate(TAPS9):
                            for kt in range(2):
                                rhs = vap(h1p[kt], s0 * 324 + ky * 18 + kx, [[324, 2], [18, 16], [1, 16]])
                                pairs.append((self.wslice(wd, f"d1c2_t{t}", kt, mt * 128), rhs))
                        self.mm(ps, pairs, dt)
                        tmp = d1.tile([128, 512], F32, tag="tmp", bufs=2, name="d1tmp")
                        nc.scalar.activation(tmp, ps, AF.Relu, bias=self.bap("d1c2_b", mt))
                        dst = vap(x1p[mt], s0 * 324 + 19, [[324, 2], [18, 16], [1, 16]])
                        nc.vector.tensor_scalar(out=w(dst),
                                                in0=tmp.rearrange("p (s y x) -> p s y x", s=2, y=16),
                                                scalar1=self.bap("d1sc_b", mt), scalar2=None, op0=ALU.add)
                    ev = vap(x1p[mt], 19, [[324, NSAMP], [36, 8], [2, 8]])
                    nc.vector.tensor_tensor(out=w(ev), in0=ev,
                                            in1=r1[mt].rearrange("p (s a b) -> p s a b", a=8, b=8),
                                            op=ALU.add)

            with contextlib.ExitStack() as ESb:
                d2 = ESb.enter_context(tc.tile_pool(name="d2p", bufs=1))
                # d2 shortcut for all samples first (reads x1p only)
                wd = self.load_w("d2a")
                for mt in range(2):
                    for nch in range(4):
                        s0 = 2 * nch
                        ps = pp.tile([128, 512], F32, tag="mm", bufs=6, name="ps_d2sc")
                        pairs = []
                        for kt in range(2):
                            rhs = vap(x1p[kt], s0 * 324 + 19, [[324, 2], [18, 16], [1, 16]])
                            pairs.append((self.wslice(wd, "d2sc", kt, mt * 128), rhs))
                        self.mm(ps, pairs, dt)
                        nc.scalar.copy(out=r2[mt][:, s0 * 256:s0 * 256 + 512], in_=ps)
                w2b = self.load_w("d2b")
                # process sample-pairs: convT1 -> h2c chunk, then conv2+logits on it
                for p2 in range(4):
                    s0 = 2 * p2
                    h2c = [d2.tile([128, 2 * 1156], F32, tag=f"h2c{kt}", bufs=2, name=f"h2c{kt}")
                           for kt in range(2)]
                    for kt in range(2):
                        nc.gpsimd.memset(h2c[kt], 0.0)
                    for mt in range(2):
                        for (py, px), taps in classes:
                            ps = pp.tile([128, 512], F32, tag="mm", bufs=6, name="ps_d2c1")
                            pairs = []
                            for t, dy, dx in taps:
                                for kt in range(2):
                                    rhs = vap(x1p[kt], s0 * 324 + (1 + dy) * 18 + 1 + dx,
                                              [[324, 2], [18, 16], [1, 16]])
                                    pairs.append((self.wslice(wd, f"d2c1_t{t}", kt, mt * 128), rhs))
                            self.mm(ps, pairs, dt)
                            dst = vap(h2c[mt], (1 + py) * 34 + 1 + px, [[1156, 2], [68, 16], [2, 16]])
                            nc.scalar.activation(w(dst), ps, AF.Relu, bias=self.bap("d2c1_b", mt))
                    for c in range(4):   # chunks within pair: (sample in pair, half)
                        sl, half = c // 2, c % 2
                        s = s0 + sl
                        r0 = 16 * half
                        xc = []
                        for mt in range(2):
                            ps = pp.tile([128, 512], F32, tag="mm", bufs=6, name="ps_d2c2")
                            pairs = []
                            for t, (ky, kx) in enumerate(TAPS9):
                                for kt in range(2):
                                    rhs = vap(h2c[kt], sl * 1156 + (ky + r0) * 34 + kx,
                                              [[34, 16], [1, 32]])
                                    pairs.append((self.wslice(w2b, f"d2c2_t{t}", kt, mt * 128), rhs))
                            self.mm(ps, pairs, dt)
                            tmp = d2.tile([128, 512], F32, tag="tmp2", bufs=2, name="d2tmp")
                            nc.scalar.activation(tmp, ps, AF.Relu, bias=self.bap("d2c2_b", mt))
                            xx = d2.tile([128, 512], F32, tag=f"xc{mt}", bufs=2, name=f"xc{mt}")
                            nc.vector.tensor_scalar(out=w(xx), in0=tmp, scalar1=self.bap("d2sc_b", mt),
                                                    scalar2=None, op0=ALU.add)
                            ev = vap(xx, 0, [[64, 8], [2, 16]])
                            rsl = vap(r2[mt], s * 256 + (r0 // 2) * 16, [[16, 8], [1, 16]])
                            nc.vector.tensor_tensor(out=w(ev), in0=ev, in1=rsl, op=ALU.add)
                            xc.append(xx)
                        ps_lg = pp.tile([16, 512], F32, tag="mm", bufs=6, name="ps_lg")
                        self.mm(ps_lg, [(self.wslice(w2b, "outw", kt, 0, 16), xc[kt])
                                        for kt in range(2)], DT_LG)
                        lg = d2.tile([16, 512], F32, tag="lg_sb", bufs=2, name="lg_sb")
                        nc.scalar.activation(lg, ps_lg[0:16, :], AF.Identity,
                                             bias=self.bb_sb[0:16, BOFF['out_b']:BOFF['out_b'] + 1])
                        cg = (p2 * 4 + c) * 4
                        for j in range(4):
                            ps_t = pp.tile([128, 16], F32, tag="mm", bufs=6, name="ps_lgt")
                            nc.tensor.transpose(ps_t, lg[0:16, j * 128:(j + 1) * 128],
                                                self.ident[0:16, 0:16])
                            nc.scalar.copy(out=out_sb[:, cg + j, :], in_=ps_t)
            dram_ap = bass.AP(tensor=self.outp, offset=0, ap=[[16, 128], [2048, 64], [1, 16]])
            nc.sync.dma_start(out=dram_ap, in_=out_sb)


# ---------------------------------------------------------------------------
# public entry
# ---------------------------------------------------------------------------

_NC_CACHE = {}


def _get_nc():
    if "nc" not in _NC_CACHE:
        _NC_CACHE["nc"] = KB().build()
    return _NC_CACHE["nc"]


def kernel(params=None, codebook=None, tokens=None, **kw):
    if params is None:
        params = kw['params']
    if codebook is None:
        codebook = kw['codebook']
    if tokens is None:
        tokens = kw['tokens']
    A = prep_arrays(params, codebook)
    wb, bb, c2 = pack_host(A)
    toks = np.ascontiguousarray(np.asarray(tokens).astype(np.int32).reshape(8, NSAMP * 1024))
    nc = _get_nc()
    in_maps = [{"wb": wb, "bb": bb, "c2": c2, "tok": toks[i][None, :]} for i in range(8)]
    res = run_bass_kernel_spmd(nc, in_maps, list(range(8)))
    out = np.concatenate([r["out"].reshape(NSAMP, 32, 32, NV) for r in res.results], axis=0)
    return out
